# revision 32
# baseline (speedup 1.0000x reference)
"""nn_BlockPositioning: out[b*8+h, i, j] = ev_h[i//4, j//4] + c_h[i%4, j%4]

with ev_h[a, b] = eb_h[a-b] if a>b else ebf_h[b-a]  (Toeplitz in a-b); the
batch axis is a pure tile of the per-head bias.  Sharding: one head per core
(8 heads, 8 cores); the 4 identical batch copies are materialized host-side
at gather time.

The per-head bias matrix is fully determined by the tiny row
  S[p, 4s+jr] = Grev[s - p//4] + c[p%4, jr],   Grev[s] = concat(eb[E-1:0:-1], ebf)
(1 MiB in bf16) via Toeplitz windowing: out[128t+p, j] = S[p, (2044-128t)+j].
The host prepares S (fp32 add, one bf16 round of the final sum - rounding
the *inputs* first would blow up rel-err where g+c nearly cancels; rounding
only the sum keeps rel err <= 2^-9 ~ 0.2% vs the 2e-2 gate), and the device
program is a pure 4-load -> 17-store DMA pipeline on one HWDGE ring:

  load S[:, 3068:4092]  -> gates store of out[0:128, 1024:2048]
  load S[:, 2044:3068]  -> gates store of out[0:128, 0:1024]
  load S[:, 1788:2044]  -> gates stores of out[128t:...] for t = 1,2
  load S[:, 0:1788]     -> gates stores of out[128t:...] for t >= 3

Each full store block is 128 contiguous 4 KiB descriptors (one per SBUF
partition = one output row) that spread over all 16 SDMA engines at line
rate (~26 GB/s each); the store phase is SDMA-engine-bound at ~21 us for
the 8 MiB head.  The loads are chunked so every store gate's completion
receipt (an HBM round trip) lands before store traffic congests HBM, and
each store is ringed >= 1.5 us before the engines reach its packets, so
the packet stream is gap-free from first load to last store (~24 us of
line-rate work for the 9 MiB moved).  SP's entry-barrier participation is
moved to the end of its stream so the first load issues as soon as the SP
sequencer boots instead of waiting for the slowest engine.  bf16 output
halves the store bytes vs fp32 (the engines are line-rate-bound per
byte); the host upcasts to fp32 at gather time.
"""

import numpy as np

_H = 8
_B = 4
_E = 512
_SEQ = 4 * _E              # 2048
_GLEN = 2 * _E - 1         # 1023
_NT = _SEQ // 128          # 16
_SEFF = 1023               # S columns s >= 1023 are never read by any window
_SROW = 4 * _SEFF          # 4092: S row length
_X0 = 4 * (_E - 1)         # 2044: window start for t=0

_CACHE = {}


def _build_nc():
    import concourse.bass as bass
    import concourse.mybir as mybir

    BF16 = mybir.dt.bfloat16
    nc = bass.Bass(enable_partition_id=False, monotonic_sem_count=0)
    s_in = nc.dram_tensor("smat", [128, _SROW], BF16, kind="ExternalInput")
    out = nc.dram_tensor("out", [_SEQ, _SEQ], BF16, kind="ExternalOutput")

    with (
        nc.sbuf_tensor([128, _SROW], BF16) as s_sb,
        nc.semaphore("d1_sem") as d1_sem,
        nc.semaphore("d2_sem") as d2_sem,
        nc.semaphore("d3_sem") as d3_sem,
        nc.semaphore("d4_sem") as d4_sem,
        nc.semaphore("ds_sem") as ds_sem,
        nc.Block() as block,
    ):
        ss = s_sb[:, :]

        # Everything runs on ONE HWDGE ring: packets drain strictly in issue
        # order, so as long as each store is issued before the SDMA engines
        # reach its packets, the stream is gap-free from the first load
        # packet to the last store packet (~23 us of line-rate work for the
        # 9 MiB moved).  Two loads: the t<=2 windows' columns first — its
        # completion receipt (an HBM round trip that gates the first three
        # stores) lands while only loads are in flight, so it isn't delayed
        # by store traffic; the second load gates t>=3, whose issue slack is
        # several us by the time the engines get there.  (Semaphore gates are
        # required: SDMA engines pipeline reads ahead of prior DMAs' writes,
        # so ring order alone does NOT give read-after-write.)
        _XB = _X0 - 256  # t=1,2 windows start at X0-128t
        _XH = _X0 + 1024  # t=0 right-half window start
        @block.sync
        def _(sync):
            # Loads in reverse window order, split so every store gate's
            # completion receipt lands before store traffic hits HBM, and
            # the first (half-width) store block is gated on a 1024-col
            # load that finishes ~2 us before the engines drain the loads.
            sync.dma_start(out=s_sb[:, _XH:], in_=s_in[:, _XH:]).then_inc(d1_sem, 16)
            sync.dma_start(
                out=s_sb[:, _XB:_XH], in_=s_in[:, _XB:_XH]
            ).then_inc(d2_sem, 16)
            sync.dma_start(out=s_sb[:, :_XB], in_=s_in[:, :_XB]).then_inc(d4_sem, 16)

            # out[128t + p, j] = S[p, (2044 - 128t) + j]; dest rows sweep
            # DRAM linearly (4 KiB writes at consecutive addresses), with a
            # 128-way outer dim that spreads over all 16 SDMA engines.
            def _store(dst, x, w=_SEQ):
                src = bass.AP(ss.tensor, ss.offset + x, [[_SROW, 128], [1, w]])
                with nc.allow_non_contiguous_dma(reason="toeplitz windows"):
                    sync.dma_start(out=dst, in_=src).then_inc(ds_sem, 16)

            sync.wait_ge(d1_sem, 16)
            _store(out[0:128, 1024:2048], _XH, 1024)
            sync.wait_ge(d2_sem, 16)
            _store(out[0:128, 0:1024], _X0, 1024)
            for t in (1, 2):
                _store(out[128 * t : 128 * (t + 1), :], _X0 - 128 * t)
            sync.wait_ge(d4_sem, 16)
            for t in range(3, _NT):
                _store(out[128 * t : 128 * (t + 1), :], _X0 - 128 * t)
            sync.wait_ge(ds_sem, 16 * (_NT + 1))

    # Move SP's entry-barrier participation (drain + inc-gather/wait-release)
    # from the preamble to the end of its DMA stream: SP then issues the
    # first load as soon as its sequencer boots (~9.5 us) instead of waiting
    # ~5 us for the slowest engine (GpSimd) to come online.  The gather
    # semaphore still reaches its target - the idle engines simply wait in
    # the barrier until SP's stores are done - so nothing deadlocks, and
    # nothing SP does depends on the other engines' preambles (the HWDGE
    # ring is runtime-configured and s_sb is written only by SP's own
    # loads).
    blocks = nc.m.functions[0].blocks
    SP = mybir.EngineType.SP
    entry, body = blocks[0], blocks[1]
    assert "_SP_" in body.name, body.name
    moved = [i for i in entry.instructions
             if i.engine == SP and type(i).__name__ in ("InstDrain", "InstEventSemaphore")]
    assert len(moved) == 2, [i.name for i in moved]
    entry.instructions = [i for i in entry.instructions if i not in moved]
    assert type(body.instructions[-1]).__name__ == "InstUnconditionalBranch"
    body.instructions = body.instructions[:-1] + moved + [body.instructions[-1]]

    return nc


def _in_maps(channel_blocks, event_blocks, event_blocks_future):
    import ml_dtypes

    maps = []
    for h in range(_H):
        eb = np.ascontiguousarray(event_blocks[:, 0, h], dtype=np.float32)
        ebf = np.ascontiguousarray(event_blocks_future[:, 0, h], dtype=np.float32)
        grev = np.concatenate([eb[_E - 1 : 0 : -1], ebf])  # (1023,)
        # row p: p//4 leading zeros, then grev (cols beyond SEFF never read)
        gs = np.zeros((128, _SEFF), dtype=np.float32)
        for q in range(32):
            n = min(_GLEN, _SEFF - q)
            gs[4 * q : 4 * q + 4, q : q + n] = grev[:n]
        c = np.ascontiguousarray(channel_blocks[:, :, 0, h], dtype=np.float32)  # (4,4)
        crow = np.tile(c, (32, 1))  # (128, 4): row p = c[p%4, :]
        s = (gs[:, :, None] + crow[:, None, :]).astype(ml_dtypes.bfloat16)
        maps.append({"smat": np.ascontiguousarray(s.reshape(128, _SROW))})
    return maps


def _compiled_runner():
    """Build (once) a jitted 8-core runner mirroring bass2jax.run_bass_via_pjrt,
    so repeat kernel() calls reuse the compiled NEFF executable."""
    if "runner" in _CACHE:
        return _CACHE["runner"]

    import jax
    import concourse.mybir as mybir
    from concourse import bass2jax
    from jax.experimental.shard_map import shard_map
    from jax.sharding import Mesh, PartitionSpec

    bass2jax.install_neuronx_cc_hook()
    if "nc" not in _CACHE:
        _CACHE["nc"] = _build_nc()
    nc = _CACHE["nc"]

    partition_name = nc.partition_id_tensor.name if nc.partition_id_tensor else None
    in_names, out_names, out_avals, zero_outs = [], [], [], []
    for alloc in nc.m.functions[0].allocations:
        if not isinstance(alloc, mybir.MemoryLocationSet):
            continue
        name = alloc.memorylocations[0].name
        if alloc.kind == "ExternalInput":
            if name != partition_name:
                in_names.append(name)
        elif alloc.kind == "ExternalOutput":
            shape = tuple(alloc.tensor_shape)
            dtype = mybir.dt.np(alloc.dtype)
            out_names.append(name)
            out_avals.append(jax.core.ShapedArray(shape, dtype))
            zero_outs.append(np.zeros(shape, dtype))
    n_params = len(in_names)
    all_in_names = in_names + out_names
    if partition_name is not None:
        all_in_names = all_in_names + [partition_name]
    all_in_names = tuple(all_in_names)

    def _body(*args):
        operands = list(args)
        if partition_name is not None:
            operands.append(bass2jax.partition_id_tensor())
        return tuple(
            bass2jax._bass_exec_p.bind(
                *operands,
                out_avals=tuple(out_avals),
                in_names=all_in_names,
                out_names=tuple(out_names),
                lowering_input_output_aliases=(),
                sim_require_finite=True,
                sim_require_nnan=True,
                nc=nc,
            )
        )

    devices = jax.devices()[:_H]
    mesh = Mesh(np.asarray(devices), ("core",))
    donate = tuple(range(n_params, n_params + len(out_names)))
    sharded = jax.jit(
        shard_map(
            _body,
            mesh=mesh,
            in_specs=(PartitionSpec("core"),) * (n_params + len(out_names)),
            out_specs=(PartitionSpec("core"),) * len(out_names),
            check_rep=False,
        ),
        donate_argnums=donate,
        keep_unused=True,
    )

    def run(in_maps):
        concat_in = [
            np.concatenate([m[name] for m in in_maps], axis=0) for name in in_names
        ]
        concat_zeros = [
            np.zeros((_H * z.shape[0], *z.shape[1:]), z.dtype) for z in zero_outs
        ]
        out_arrs = sharded(*concat_in, *concat_zeros)
        return [
            {
                name: np.asarray(out_arrs[i]).reshape(_H, *out_avals[i].shape)[c]
                for i, name in enumerate(out_names)
            }
            for c in range(_H)
        ]

    _CACHE["runner"] = run
    return run


def run_spmd(channel_blocks, event_blocks, event_blocks_future):
    """Run the per-head kernels on cores 0-7; returns (None, heads).

    heads: bfloat16 (8, 2048, 2048), one bias matrix per head."""
    run = _compiled_runner()
    results = run(_in_maps(channel_blocks, event_blocks, event_blocks_future))
    heads = np.stack([np.asarray(results[h]["out"]) for h in range(_H)])
    return None, heads


def kernel(q, channel_blocks, event_blocks, event_blocks_future):
    q = np.asarray(q)
    channel_blocks = np.asarray(channel_blocks, dtype=np.float32)
    event_blocks = np.asarray(event_blocks, dtype=np.float32)
    event_blocks_future = np.asarray(event_blocks_future, dtype=np.float32)

    _, heads = run_spmd(channel_blocks, event_blocks, event_blocks_future)
    batch = q.shape[0] // _H
    return np.tile(heads.astype(np.float32), (batch, 1, 1))
